# revision 19
# baseline (speedup 1.0000x reference)
"""GNN (2x SAGEConv + linear) Bass kernel for trn2, 8 NeuronCores.

Sharding: nodes partitioned across 8 cores (12500 each, dst-range).
Each layer: per-core windowed padded-CSR gathers of h[src] (dma_gather over
4 SWDGE queues), on-chip segment reduce (DVE strided), batched unique-row
dma_scatter_add into per-window DRAM accumulators, dense combine + PE MLP.
The inter-layer AllGather of h1 slices is split into 4 quarter collectives
that pipeline with phase 2 and with layer 2's first window gathers (layer-2
gather windows are quarter-of-every-core tensors).
"""
import numpy as np

N = 100000
E = 1250000
HID = 64
P = 8
NPC = 12500          # nodes per core
RPC = 12544          # rows per core block (98 * 128), rows 12500+ are zero pads
NB = RPC // 128      # 98 blocks
WIN = 2 * RPC        # 25088 rows per L1 gather window (2 emb blocks)
NW = 4               # windows per layer
ZLOC = 12500         # local row inside an L1 window that is guaranteed zero
ACCR = RPC + 128     # accumulator rows (tail rows are scratch)
MAXPOS = 2048        # max gather positions per call
MAXT = 16            # max tiles per gather call

# layer-2 quarter windows: per-core h1 slice split at 128-node block bounds
QOFF = [0, 3200, 6400, 9472]           # node offsets within a core slice
QSZ = [3200, 3200, 3072, 3072]         # real rows per quarter
QSZP = [q + 16 for q in QSZ]           # +16 zero pad rows per core
QB0 = [0, 25, 50, 74]                  # first 128-node block of each quarter
QB1 = [25, 50, 74, 98]                 # one-past-last block


def _wrap128(vals):
    """flat int16 stream -> [128, len/16] wrapped+replicated layout."""
    n = vals.shape[0]
    w16 = np.ascontiguousarray(vals.reshape(n // 16, 16).T)
    return np.tile(w16, (8, 1))


def _build_layer_meta(w_of, loc, dst, wrows, zpad):
    """Per-layer gather/scatter metadata.

    w_of: window of each edge's source [E]
    loc:  row of the source within its window [E]
    dst:  destination node per edge [E]
    wrows: rows per window (bounds check)
    zpad: per-window row index that is guaranteed zero (pad target)
    """
    core = dst // NPC
    dstl = dst % NPC

    deg = np.zeros((P, NW, RPC), np.int32)
    np.add.at(deg, (core, w_of, dstl), 1)

    order = np.argsort(-deg, axis=2, kind="stable")  # [P, NW, RPC]
    deg_sorted = -np.sort(-deg, axis=2)
    tile_max = deg_sorted.reshape(P, NW, NB, 128).max(axis=3)
    D = tile_max.max(axis=0)                         # [NW, NB] shared

    groups = []
    for w in range(NW):
        gw = []
        cur, curpos = [], 0
        for t in range(NB):
            d = int(D[w, t])
            if d == 0:
                continue
            if cur and (curpos + d * 128 > MAXPOS or len(cur) >= MAXT):
                gw.append(cur)
                cur, curpos = [], 0
            cur.append(t)
            curpos += d * 128
        if cur:
            gw.append(cur)
        groups.append(gw)

    eorder = np.lexsort((loc, dstl, w_of, core))
    sc, sw, sd, sl = core[eorder], w_of[eorder], dstl[eorder], loc[eorder]
    key = ((sc * NW + sw) * RPC + sd).astype(np.int64)
    starts = np.searchsorted(key, np.arange(P * NW * RPC, dtype=np.int64))
    starts = np.append(starts, len(key))

    for w in range(NW):
        assert loc[w_of == w].max(initial=0) < wrows[w] <= 32767

    gidx_cores, sidx_cores = [], []
    for k in range(P):
        gparts, sparts = [], []
        for w in range(NW):
            od = order[k, w]
            for gt in groups[w]:
                for t in gt:
                    d = int(D[w, t])
                    nodes = od[t * 128:(t + 1) * 128]
                    blockg = np.full((d, 128), zpad[w], np.int32)
                    for p in range(128):
                        nloc = int(nodes[p])
                        s0 = starts[(k * NW + w) * RPC + nloc]
                        s1 = starts[(k * NW + w) * RPC + nloc + 1]
                        cnt = s1 - s0
                        if cnt:
                            blockg[:cnt, p] = sl[s0:s1]
                    gparts.append(blockg.reshape(-1))
                srows = np.concatenate(
                    [od[t * 128:(t + 1) * 128] for t in gt]).astype(np.int32)
                sparts.append(srows)
        gidx_cores.append(_wrap128(np.concatenate(gparts).astype(np.int16)))
        sidx_cores.append(_wrap128(np.concatenate(sparts).astype(np.int16)))
    return groups, D, gidx_cores, sidx_cores


def kernel(x, edge_index, edge_weight, emb, Wl1, bl1, Wr1, a1,
           Wl2, bl2, Wr2, a2, Wout, bout):
    import concourse.bacc as bacc
    import concourse.mybir as mybir
    import concourse.tile as tile
    from concourse.bass_utils import run_bass_kernel_spmd
    from concourse.masks import make_identity

    x = np.asarray(x).astype(np.int64)
    ei = np.asarray(edge_index).astype(np.int64)
    emb = np.asarray(emb, np.float32)
    Wl1 = np.asarray(Wl1, np.float32); Wr1 = np.asarray(Wr1, np.float32)
    Wl2 = np.asarray(Wl2, np.float32); Wr2 = np.asarray(Wr2, np.float32)
    Wout = np.asarray(Wout, np.float32)
    bl1 = np.asarray(bl1, np.float32); bl2 = np.asarray(bl2, np.float32)
    bout = np.asarray(bout, np.float32)
    a1f = float(np.asarray(a1)); a2f = float(np.asarray(a2))
    src, dst = ei[0], ei[1]

    # ---- host prep ------------------------------------------------------
    emb_hc = np.zeros((P * RPC, HID), np.float32)
    for r in range(P):
        emb_hc[r * RPC:r * RPC + NPC] = emb[r * NPC:(r + 1) * NPC]

    h0_own = np.zeros((P, RPC, HID), np.float32)
    for k in range(P):
        h0_own[k, :NPC] = emb[x[k * NPC:(k + 1) * NPC]]

    cnt = np.bincount(dst, minlength=N).astype(np.float32)
    invc = np.zeros((P, 128, NB), np.float32)
    for k in range(P):
        c = np.zeros(RPC, np.float32)
        c[:NPC] = 1.0 / np.maximum(cnt[k * NPC:(k + 1) * NPC], 1.0)
        invc[k] = c.reshape(NB, 128).T

    # L1: sources are emb rows in hcat layout (two 12544 blocks per window)
    sid1 = x[src]
    w1 = sid1 // (2 * NPC)
    loc1 = RPC * ((sid1 // NPC) % 2) + sid1 % NPC
    g1, D1, gidx1, sidx1 = _build_layer_meta(
        w1, loc1, dst, [WIN] * 4, [ZLOC] * 4)

    # L2: sources are h1 rows in quarter-window layout
    k2 = src // NPC
    r2 = src % NPC
    q2 = np.digitize(r2, QOFF[1:])               # quarter index 0..3
    qoff = np.array(QOFF)[q2]
    qszp = np.array(QSZP)[q2]
    loc2 = k2 * qszp + (r2 - qoff)
    g2, D2, gidx2, sidx2 = _build_layer_meta(
        q2, loc2, dst, [8 * s for s in QSZP], list(QSZ))

    # ---- device program -------------------------------------------------
    f32, i16 = mybir.dt.float32, mybir.dt.int16
    nc = bacc.Bacc(dynamic_dma_scratch_size=65536, num_swdge_queues=4)
    dp = nc.declare_dram_parameter
    embw = dp("embw", [P * RPC, HID], f32, isOutput=False)
    h0o = dp("h0o", [RPC, HID], f32, isOutput=False)
    gi1 = dp("gi1", list(gidx1[0].shape), i16, isOutput=False)
    si1 = dp("si1", list(sidx1[0].shape), i16, isOutput=False)
    gi2 = dp("gi2", list(gidx2[0].shape), i16, isOutput=False)
    si2 = dp("si2", list(sidx2[0].shape), i16, isOutput=False)
    invc_p = dp("invc", [128, NB], f32, isOutput=False)
    wl1_p = dp("wl1", [HID, HID], f32, isOutput=False)
    wr1_p = dp("wr1", [HID, HID], f32, isOutput=False)
    wl2_p = dp("wl2", [HID, HID], f32, isOutput=False)
    wr2_p = dp("wr2", [HID, HID], f32, isOutput=False)
    wout_p = dp("wout", [HID, HID], f32, isOutput=False)
    bl1_p = dp("bl1t", [HID, 1], f32, isOutput=False)
    bl2_p = dp("bl2t", [HID, 1], f32, isOutput=False)
    bout_p = dp("boutr", [128, HID], f32, isOutput=False)
    out_p = dp("out", [RPC, HID], f32, isOutput=True)

    acc_d = [nc.dram_tensor(f"acc{w}", [ACCR, HID], f32) for w in range(NW)]
    hq_in = [nc.dram_tensor(f"hqi{q}", [QSZP[q], HID], f32) for q in range(4)]
    hq_out = [nc.dram_tensor(f"hqo{q}", [P * QSZP[q], HID], f32,
                             addr_space="Shared") for q in range(4)]

    AX = mybir.AxisListType.X
    ADD = mybir.AluOpType.add
    PRELU = mybir.ActivationFunctionType.Prelu

    with tile.TileContext(nc) as tc:
        with tc.tile_pool(name="const", bufs=1) as cpool, \
             tc.tile_pool(name="big", bufs=1) as bpool, \
             tc.tile_pool(name="gio", bufs=3) as gpool, \
             tc.tile_pool(name="ph2", bufs=3) as qpool, \
             tc.tile_pool(name="ps", bufs=1, space="PSUM") as ppool:

            ident = cpool.tile([128, 128], f32)
            make_identity(nc, ident[:])
            wl1_t = cpool.tile([HID, HID], f32); nc.sync.dma_start(wl1_t[:], wl1_p[:])
            wr1_t = cpool.tile([HID, HID], f32); nc.sync.dma_start(wr1_t[:], wr1_p[:])
            wl2_t = cpool.tile([HID, HID], f32); nc.sync.dma_start(wl2_t[:], wl2_p[:])
            wr2_t = cpool.tile([HID, HID], f32); nc.sync.dma_start(wr2_t[:], wr2_p[:])
            wout_t = cpool.tile([HID, HID], f32); nc.sync.dma_start(wout_t[:], wout_p[:])
            bl1_t = cpool.tile([HID, 1], f32); nc.sync.dma_start(bl1_t[:], bl1_p[:])
            bl2_t = cpool.tile([HID, 1], f32); nc.sync.dma_start(bl2_t[:], bl2_p[:])
            bout_t = cpool.tile([128, HID], f32); nc.sync.dma_start(bout_t[:], bout_p[:])
            invc_t = cpool.tile([128, NB], f32); nc.sync.dma_start(invc_t[:], invc_p[:])

            h1T = bpool.tile([HID, NB, 128], f32)      # h1 transposed, own nodes
            hc1_t = bpool.tile([128, NB, HID], f32)    # h1 node-major, own nodes
            zt = cpool.tile([128, HID], f32)
            nc.vector.memset(zt[:], 0.0)
            zbig = cpool.tile([128, 33, HID], f32)
            nc.vector.memset(zbig[:], 0.0)

            # zero pad rows of the quarter collective inputs (once)
            for q in range(4):
                nc.sync.dma_start(hq_in[q][QSZ[q]:QSZP[q]], zt[:16, :])

            def zero_accs():
                for w in range(NW):
                    dstv = acc_d[w][:].rearrange("(r p) f -> p r f", p=128)
                    for c in range(3):
                        nc.sync.dma_start(dstv[:, c * 33:(c + 1) * 33, :], zbig[:])

            def phase1(groups, D, gi_p, si_p, win_aps):
                gi_t = bpool.tile([128, gi_p.shape[1]], i16, tag="gi")
                si_t = bpool.tile([128, si_p.shape[1]], i16, tag="si")
                nc.sync.dma_start(gi_t[:], gi_p[:])
                nc.sync.dma_start(si_t[:], si_p[:])
                gcol = 0
                scol = 0
                qn = 0
                for w in range(NW):
                    win = win_aps[w]
                    for gt in groups[w]:
                        npos = int(sum(D[w, t] for t in gt)) * 128
                        ncols = npos // 128
                        nt = len(gt)
                        g_t = gpool.tile([128, MAXPOS // 128, HID], f32, tag="g")
                        r_t = gpool.tile([128, MAXT, HID], f32, tag="r")
                        nc.gpsimd.dma_gather(
                            g_t[:, :ncols, :], win, gi_t[:, gcol:gcol + npos // 16],
                            npos, npos, HID, single_packet=False,
                            queue_num=qn % 4)
                        off = 0
                        for i, t in enumerate(gt):
                            d = int(D[w, t])
                            view = g_t[:, off:off + d, :].rearrange("p d f -> p f d")
                            nc.vector.tensor_reduce(r_t[:, i, :], view, axis=AX, op=ADD)
                            off += d
                        nc.gpsimd.dma_scatter_add(
                            acc_d[w][:], r_t[:, :nt, :], si_t[:, scol:scol + nt * 8],
                            nt * 128, nt * 128, HID, single_packet=False,
                            queue_num=(qn + 2) % 4)
                        gcol += npos // 16
                        scol += nt * 8
                        qn += 1

            def phase2(L):
                wl_t = wl1_t if L == 1 else wl2_t
                wr_t = wr1_t if L == 1 else wr2_t
                bl_t = bl1_t if L == 1 else bl2_t
                alpha = a1f if L == 1 else a2f
                for b in range(NB):
                    m_t = qpool.tile([128, NW, HID], f32, tag="m")
                    for w in range(NW):
                        nc.sync.dma_start(m_t[:, w, :],
                                          acc_d[w][b * 128:(b + 1) * 128])
                    mean0 = qpool.tile([128, HID], f32, tag="mean0")
                    nc.vector.tensor_reduce(
                        mean0[:], m_t[:].rearrange("p w f -> p f w"), axis=AX, op=ADD)
                    meansc = qpool.tile([128, HID], f32, tag="meansc")
                    nc.vector.tensor_scalar_mul(meansc[:], mean0[:], invc_t[:, b:b + 1])
                    psA = ppool.tile([HID, 128], f32, tag="psA")
                    nc.tensor.transpose(psA[:], meansc[:], ident[:])
                    meanT = qpool.tile([HID, 128], f32, tag="meanT")
                    nc.vector.tensor_copy(meanT[:], psA[:])
                    if L == 1:
                        hob = qpool.tile([128, HID], f32, tag="hob")
                        nc.sync.dma_start(hob[:], h0o[b * 128:(b + 1) * 128])
                        psB = ppool.tile([HID, 128], f32, tag="psB")
                        nc.tensor.transpose(psB[:], hob[:], ident[:])
                        hT = qpool.tile([HID, 128], f32, tag="hT")
                        nc.vector.tensor_copy(hT[:], psB[:])
                        hT_ap = hT[:]
                    else:
                        hT_ap = h1T[:, b, :]
                    psC = ppool.tile([HID, 128], f32, tag="psC")
                    nc.tensor.matmul(psC[:], wl_t[:], meanT[:], start=True, stop=False)
                    nc.tensor.matmul(psC[:], wr_t[:], hT_ap, start=False, stop=True)
                    if L == 1:
                        nc.scalar.activation(h1T[:, b, :], psC[:], PRELU,
                                             bias=bl_t[:], alpha=alpha)
                        psD = ppool.tile([128, HID], f32, tag="psD")
                        nc.tensor.transpose(psD[:], h1T[:, b, :], ident[:HID, :HID])
                        nc.vector.tensor_copy(hc1_t[:, b, :], psD[:])
                        for q in range(4):
                            if b == QB1[q] - 1:
                                nc.sync.dma_start(
                                    hq_in[q][0:QSZ[q]].rearrange(
                                        "(r p) f -> p r f", p=128),
                                    hc1_t[:, QB0[q]:QB1[q], :])
                                if q == 3:
                                    # zero h1 pad rows (nodes 12500..12543)
                                    nc.sync.dma_start(
                                        hq_in[3][NPC - QOFF[3]:QSZ[3]],
                                        zt[:QSZ[3] - (NPC - QOFF[3]), :])
                                nc.gpsimd.collective_compute(
                                    "AllGather", mybir.AluOpType.bypass,
                                    replica_groups=[list(range(P))],
                                    ins=[hq_in[q][:]], outs=[hq_out[q][:]])
                    else:
                        h2T = qpool.tile([HID, 128], f32, tag="h2T")
                        nc.scalar.activation(h2T[:], psC[:], PRELU,
                                             bias=bl_t[:], alpha=alpha)
                        psE = ppool.tile([128, HID], f32, tag="psE")
                        nc.tensor.matmul(psE[:], h2T[:], wout_t[:], start=True, stop=True)
                        outb = qpool.tile([128, HID], f32, tag="outb")
                        nc.vector.tensor_tensor(outb[:], psE[:], bout_t[:], op=ADD)
                        nc.sync.dma_start(out_p[b * 128:(b + 1) * 128], outb[:])

            # ---- layer 1 ----
            zero_accs()
            phase1(g1, D1, gi1, si1,
                   [embw[w * WIN:(w + 1) * WIN] for w in range(NW)])
            phase2(1)
            # zero pad rows of h1T (nodes 12500..12543) used by L2's Wr term
            nc.vector.memset(h1T[:, NB - 1, 84:128], 0.0)
            # ---- layer 2 + out ----
            zero_accs()
            phase1(g2, D2, gi2, si2, [hq_out[q][:] for q in range(4)])
            phase2(2)

    nc.compile()

    in_maps = []
    for k in range(P):
        in_maps.append({
            "embw": emb_hc, "h0o": h0_own[k],
            "gi1": gidx1[k], "si1": sidx1[k],
            "gi2": gidx2[k], "si2": sidx2[k],
            "invc": invc[k],
            "wl1": Wl1, "wr1": Wr1, "wl2": Wl2, "wr2": Wr2, "wout": Wout,
            "bl1t": bl1.reshape(HID, 1), "bl2t": bl2.reshape(HID, 1),
            "boutr": np.tile(bout.reshape(1, HID), (128, 1)),
        })
    res = run_bass_kernel_spmd(nc, in_maps, list(range(P)))
    out = np.zeros((N, HID), np.float32)
    for k in range(P):
        out[k * NPC:(k + 1) * NPC] = res.results[k]["out"][:NPC]
    kernel.last_exec_time_ns = res.exec_time_ns
    return out


# revision 20
# speedup vs baseline: 1.1108x; 1.1108x over previous
"""GNN (2x SAGEConv + linear) Bass kernel for trn2, 8 NeuronCores.

Sharding: nodes partitioned across 8 cores (12500 each, dst-range).
Each layer: per-core windowed padded-CSR gathers of h[src] (dma_gather over
4 SWDGE queues), on-chip segment reduce (DVE strided), batched unique-row
dma_scatter_add into per-window DRAM accumulators, dense combine + PE MLP.
The inter-layer AllGather of h1 slices is split into 4 quarter collectives
that pipeline with phase 2 and with layer 2's first window gathers (layer-2
gather windows are quarter-of-every-core tensors).
"""
import numpy as np

N = 100000
E = 1250000
HID = 64
P = 8
NPC = 12500          # nodes per core
RPC = 12544          # rows per core block (98 * 128), rows 12500+ are zero pads
NB = RPC // 128      # 98 blocks
WIN = 2 * RPC        # 25088 rows per L1 gather window (2 emb blocks)
NW = 4               # windows per layer
ZLOC = 12500         # local row inside an L1 window that is guaranteed zero
ACCR = RPC + 128     # accumulator rows (tail rows are scratch)
MAXPOS = 2048        # max gather positions per call
MAXT = 16            # max tiles per gather call

# layer-2 quarter windows: per-core h1 slice split at 128-node block bounds
QOFF = [0, 3200, 6400, 9472]           # node offsets within a core slice
QSZ = [3200, 3200, 3072, 3072]         # real rows per quarter
QSZP = [q + 16 for q in QSZ]           # +16 zero pad rows per core
QB0 = [0, 25, 50, 74]                  # first 128-node block of each quarter
QB1 = [25, 50, 74, 98]                 # one-past-last block


def _wrap128(vals):
    """flat int16 stream -> [128, len/16] wrapped+replicated layout."""
    n = vals.shape[0]
    w16 = np.ascontiguousarray(vals.reshape(n // 16, 16).T)
    return np.tile(w16, (8, 1))


def _build_layer_meta(w_of, loc, dst, wrows, zpad):
    """Per-layer gather/scatter metadata.

    w_of: window of each edge's source [E]
    loc:  row of the source within its window [E]
    dst:  destination node per edge [E]
    wrows: rows per window (bounds check)
    zpad: per-window row index that is guaranteed zero (pad target)
    """
    core = dst // NPC
    dstl = dst % NPC

    deg = np.zeros((P, NW, RPC), np.int32)
    np.add.at(deg, (core, w_of, dstl), 1)

    order = np.argsort(-deg, axis=2, kind="stable")  # [P, NW, RPC]
    deg_sorted = -np.sort(-deg, axis=2)
    tile_max = deg_sorted.reshape(P, NW, NB, 128).max(axis=3)
    D = tile_max.max(axis=0)                         # [NW, NB] shared

    groups = []
    for w in range(NW):
        gw = []
        cur, curpos = [], 0
        for t in range(NB):
            d = int(D[w, t])
            if d == 0:
                continue
            if cur and (curpos + d * 128 > MAXPOS or len(cur) >= MAXT):
                gw.append(cur)
                cur, curpos = [], 0
            cur.append(t)
            curpos += d * 128
        if cur:
            gw.append(cur)
        groups.append(gw)

    eorder = np.lexsort((loc, dstl, w_of, core))
    sc, sw, sd, sl = core[eorder], w_of[eorder], dstl[eorder], loc[eorder]
    key = ((sc * NW + sw) * RPC + sd).astype(np.int64)
    starts = np.searchsorted(key, np.arange(P * NW * RPC, dtype=np.int64))
    starts = np.append(starts, len(key))

    for w in range(NW):
        assert loc[w_of == w].max(initial=0) < wrows[w] <= 32767

    gidx_cores, sidx_cores = [], []
    for k in range(P):
        gparts, sparts = [], []
        for w in range(NW):
            od = order[k, w]
            for gt in groups[w]:
                for t in gt:
                    d = int(D[w, t])
                    nodes = od[t * 128:(t + 1) * 128]
                    blockg = np.full((d, 128), zpad[w], np.int32)
                    for p in range(128):
                        nloc = int(nodes[p])
                        s0 = starts[(k * NW + w) * RPC + nloc]
                        s1 = starts[(k * NW + w) * RPC + nloc + 1]
                        cnt = s1 - s0
                        if cnt:
                            blockg[:cnt, p] = sl[s0:s1]
                    gparts.append(blockg.reshape(-1))
                srows = np.concatenate(
                    [od[t * 128:(t + 1) * 128] for t in gt]).astype(np.int32)
                sparts.append(srows)
        gidx_cores.append(_wrap128(np.concatenate(gparts).astype(np.int16)))
        sidx_cores.append(_wrap128(np.concatenate(sparts).astype(np.int16)))
    return groups, D, gidx_cores, sidx_cores


def kernel(x, edge_index, edge_weight, emb, Wl1, bl1, Wr1, a1,
           Wl2, bl2, Wr2, a2, Wout, bout):
    import concourse.bacc as bacc
    import concourse.mybir as mybir
    import concourse.tile as tile
    from concourse.bass_utils import run_bass_kernel_spmd
    from concourse.masks import make_identity

    x = np.asarray(x).astype(np.int64)
    ei = np.asarray(edge_index).astype(np.int64)
    emb = np.asarray(emb, np.float32)
    Wl1 = np.asarray(Wl1, np.float32); Wr1 = np.asarray(Wr1, np.float32)
    Wl2 = np.asarray(Wl2, np.float32); Wr2 = np.asarray(Wr2, np.float32)
    Wout = np.asarray(Wout, np.float32)
    bl1 = np.asarray(bl1, np.float32); bl2 = np.asarray(bl2, np.float32)
    bout = np.asarray(bout, np.float32)
    a1f = float(np.asarray(a1)); a2f = float(np.asarray(a2))
    src, dst = ei[0], ei[1]

    # ---- host prep ------------------------------------------------------
    emb_hc = np.zeros((P * RPC, HID), np.float32)
    for r in range(P):
        emb_hc[r * RPC:r * RPC + NPC] = emb[r * NPC:(r + 1) * NPC]

    h0_own = np.zeros((P, RPC, HID), np.float32)
    for k in range(P):
        h0_own[k, :NPC] = emb[x[k * NPC:(k + 1) * NPC]]

    cnt = np.bincount(dst, minlength=N).astype(np.float32)
    invc = np.zeros((P, 128, NB), np.float32)
    for k in range(P):
        c = np.zeros(RPC, np.float32)
        c[:NPC] = 1.0 / np.maximum(cnt[k * NPC:(k + 1) * NPC], 1.0)
        invc[k] = c.reshape(NB, 128).T

    # L1: sources are emb rows in hcat layout (two 12544 blocks per window)
    sid1 = x[src]
    w1 = sid1 // (2 * NPC)
    loc1 = RPC * ((sid1 // NPC) % 2) + sid1 % NPC
    g1, D1, gidx1, sidx1 = _build_layer_meta(
        w1, loc1, dst, [WIN] * 4, [ZLOC] * 4)

    # L2: sources are h1 rows in quarter-window layout
    k2 = src // NPC
    r2 = src % NPC
    q2 = np.digitize(r2, QOFF[1:])               # quarter index 0..3
    qoff = np.array(QOFF)[q2]
    qszp = np.array(QSZP)[q2]
    loc2 = k2 * qszp + (r2 - qoff)
    g2, D2, gidx2, sidx2 = _build_layer_meta(
        q2, loc2, dst, [8 * s for s in QSZP], list(QSZ))

    # ---- device program -------------------------------------------------
    f32, i16 = mybir.dt.float32, mybir.dt.int16
    nc = bacc.Bacc(dynamic_dma_scratch_size=65536, num_swdge_queues=4)
    dp = nc.declare_dram_parameter
    embw = dp("embw", [P * RPC, HID], f32, isOutput=False)
    h0o = dp("h0o", [RPC, HID], f32, isOutput=False)
    gi1 = dp("gi1", list(gidx1[0].shape), i16, isOutput=False)
    si1 = dp("si1", list(sidx1[0].shape), i16, isOutput=False)
    gi2 = dp("gi2", list(gidx2[0].shape), i16, isOutput=False)
    si2 = dp("si2", list(sidx2[0].shape), i16, isOutput=False)
    invc_p = dp("invc", [128, NB], f32, isOutput=False)
    wl1_p = dp("wl1", [HID, HID], f32, isOutput=False)
    wr1_p = dp("wr1", [HID, HID], f32, isOutput=False)
    wl2_p = dp("wl2", [HID, HID], f32, isOutput=False)
    wr2_p = dp("wr2", [HID, HID], f32, isOutput=False)
    wout_p = dp("wout", [HID, HID], f32, isOutput=False)
    bl1_p = dp("bl1t", [HID, 1], f32, isOutput=False)
    bl2_p = dp("bl2t", [HID, 1], f32, isOutput=False)
    bout_p = dp("boutr", [128, HID], f32, isOutput=False)
    out_p = dp("out", [RPC, HID], f32, isOutput=True)

    acc_d1 = [nc.dram_tensor(f"acc{w}", [ACCR, HID], f32) for w in range(NW)]
    acc_d2 = [nc.dram_tensor(f"accB{w}", [ACCR, HID], f32) for w in range(NW)]
    hq_in = [nc.dram_tensor(f"hqi{q}", [QSZP[q], HID], f32) for q in range(4)]
    hq_out = [nc.dram_tensor(f"hqo{q}", [P * QSZP[q], HID], f32,
                             addr_space="Shared") for q in range(4)]

    AX = mybir.AxisListType.X
    ADD = mybir.AluOpType.add
    PRELU = mybir.ActivationFunctionType.Prelu

    with tile.TileContext(nc) as tc:
        with tc.tile_pool(name="const", bufs=1) as cpool, \
             tc.tile_pool(name="big", bufs=1) as bpool, \
             tc.tile_pool(name="gio", bufs=3) as gpool, \
             tc.tile_pool(name="ph2", bufs=3) as qpool, \
             tc.tile_pool(name="ps", bufs=1, space="PSUM") as ppool:

            ident = cpool.tile([128, 128], f32)
            make_identity(nc, ident[:])
            wl1_t = cpool.tile([HID, HID], f32); nc.sync.dma_start(wl1_t[:], wl1_p[:])
            wr1_t = cpool.tile([HID, HID], f32); nc.sync.dma_start(wr1_t[:], wr1_p[:])
            wl2_t = cpool.tile([HID, HID], f32); nc.sync.dma_start(wl2_t[:], wl2_p[:])
            wr2_t = cpool.tile([HID, HID], f32); nc.sync.dma_start(wr2_t[:], wr2_p[:])
            wout_t = cpool.tile([HID, HID], f32); nc.sync.dma_start(wout_t[:], wout_p[:])
            bl1_t = cpool.tile([HID, 1], f32); nc.sync.dma_start(bl1_t[:], bl1_p[:])
            bl2_t = cpool.tile([HID, 1], f32); nc.sync.dma_start(bl2_t[:], bl2_p[:])
            bout_t = cpool.tile([128, HID], f32); nc.sync.dma_start(bout_t[:], bout_p[:])
            invc_t = cpool.tile([128, NB], f32); nc.sync.dma_start(invc_t[:], invc_p[:])

            h1T = bpool.tile([HID, NB, 128], f32)      # h1 transposed, own nodes
            hc1_t = bpool.tile([128, NB, HID], f32)    # h1 node-major, own nodes
            zt = cpool.tile([128, HID], f32)
            nc.vector.memset(zt[:], 0.0)
            zbig = cpool.tile([128, 33, HID], f32)
            nc.vector.memset(zbig[:], 0.0)

            # zero pad rows of the quarter collective inputs (once)
            for q in range(4):
                nc.sync.dma_start(hq_in[q][QSZ[q]:QSZP[q]], zt[:16, :])

            def zero_accs(acc_d):
                for w in range(NW):
                    dstv = acc_d[w][:].rearrange("(r p) f -> p r f", p=128)
                    for c in range(3):
                        nc.sync.dma_start(dstv[:, c * 33:(c + 1) * 33, :], zbig[:])

            def phase1(groups, D, gi_p, si_p, win_aps, acc_d):
                gi_t = bpool.tile([128, gi_p.shape[1]], i16, tag="gi")
                si_t = bpool.tile([128, si_p.shape[1]], i16, tag="si")
                nc.sync.dma_start(gi_t[:], gi_p[:])
                nc.sync.dma_start(si_t[:], si_p[:])
                gcol = 0
                scol = 0
                qn = 0
                for w in range(NW):
                    win = win_aps[w]
                    for gt in groups[w]:
                        npos = int(sum(D[w, t] for t in gt)) * 128
                        ncols = npos // 128
                        nt = len(gt)
                        g_t = gpool.tile([128, MAXPOS // 128, HID], f32, tag="g")
                        r_t = gpool.tile([128, MAXT, HID], f32, tag="r")
                        nc.gpsimd.dma_gather(
                            g_t[:, :ncols, :], win, gi_t[:, gcol:gcol + npos // 16],
                            npos, npos, HID, single_packet=False,
                            queue_num=qn % 3)
                        off = 0
                        for i, t in enumerate(gt):
                            d = int(D[w, t])
                            view = g_t[:, off:off + d, :].rearrange("p d f -> p f d")
                            nc.vector.tensor_reduce(r_t[:, i, :], view, axis=AX, op=ADD)
                            off += d
                        nc.gpsimd.dma_scatter_add(
                            acc_d[w][:], r_t[:, :nt, :], si_t[:, scol:scol + nt * 8],
                            nt * 128, nt * 128, HID, single_packet=False,
                            queue_num=3)
                        gcol += npos // 16
                        scol += nt * 8
                        qn += 1

            def phase2(L, acc_d):
                wl_t = wl1_t if L == 1 else wl2_t
                wr_t = wr1_t if L == 1 else wr2_t
                bl_t = bl1_t if L == 1 else bl2_t
                alpha = a1f if L == 1 else a2f
                for b in range(NB):
                    m_t = qpool.tile([128, NW, HID], f32, tag="m")
                    for w in range(NW):
                        nc.sync.dma_start(m_t[:, w, :],
                                          acc_d[w][b * 128:(b + 1) * 128])
                    mean0 = qpool.tile([128, HID], f32, tag="mean0")
                    nc.vector.tensor_reduce(
                        mean0[:], m_t[:].rearrange("p w f -> p f w"), axis=AX, op=ADD)
                    meansc = qpool.tile([128, HID], f32, tag="meansc")
                    nc.vector.tensor_scalar_mul(meansc[:], mean0[:], invc_t[:, b:b + 1])
                    psA = ppool.tile([HID, 128], f32, tag="psA")
                    nc.tensor.transpose(psA[:], meansc[:], ident[:])
                    meanT = qpool.tile([HID, 128], f32, tag="meanT")
                    nc.vector.tensor_copy(meanT[:], psA[:])
                    if L == 1:
                        hob = qpool.tile([128, HID], f32, tag="hob")
                        nc.sync.dma_start(hob[:], h0o[b * 128:(b + 1) * 128])
                        psB = ppool.tile([HID, 128], f32, tag="psB")
                        nc.tensor.transpose(psB[:], hob[:], ident[:])
                        hT = qpool.tile([HID, 128], f32, tag="hT")
                        nc.vector.tensor_copy(hT[:], psB[:])
                        hT_ap = hT[:]
                    else:
                        hT_ap = h1T[:, b, :]
                    psC = ppool.tile([HID, 128], f32, tag="psC")
                    nc.tensor.matmul(psC[:], wl_t[:], meanT[:], start=True, stop=False)
                    nc.tensor.matmul(psC[:], wr_t[:], hT_ap, start=False, stop=True)
                    if L == 1:
                        nc.scalar.activation(h1T[:, b, :], psC[:], PRELU,
                                             bias=bl_t[:], alpha=alpha)
                        psD = ppool.tile([128, HID], f32, tag="psD")
                        nc.tensor.transpose(psD[:], h1T[:, b, :], ident[:HID, :HID])
                        nc.vector.tensor_copy(hc1_t[:, b, :], psD[:])
                        for q in range(4):
                            if b == QB1[q] - 1:
                                nc.sync.dma_start(
                                    hq_in[q][0:QSZ[q]].rearrange(
                                        "(r p) f -> p r f", p=128),
                                    hc1_t[:, QB0[q]:QB1[q], :])
                                if q == 3:
                                    # zero h1 pad rows (nodes 12500..12543)
                                    nc.sync.dma_start(
                                        hq_in[3][NPC - QOFF[3]:QSZ[3]],
                                        zt[:QSZ[3] - (NPC - QOFF[3]), :])
                                nc.gpsimd.collective_compute(
                                    "AllGather", mybir.AluOpType.bypass,
                                    replica_groups=[list(range(P))],
                                    ins=[hq_in[q][:]], outs=[hq_out[q][:]])
                    else:
                        h2T = qpool.tile([HID, 128], f32, tag="h2T")
                        nc.scalar.activation(h2T[:], psC[:], PRELU,
                                             bias=bl_t[:], alpha=alpha)
                        psE = ppool.tile([128, HID], f32, tag="psE")
                        nc.tensor.matmul(psE[:], h2T[:], wout_t[:], start=True, stop=True)
                        outb = qpool.tile([128, HID], f32, tag="outb")
                        nc.vector.tensor_tensor(outb[:], psE[:], bout_t[:], op=ADD)
                        nc.sync.dma_start(out_p[b * 128:(b + 1) * 128], outb[:])

            # ---- layer 1 ----
            zero_accs(acc_d1)
            zero_accs(acc_d2)
            phase1(g1, D1, gi1, si1,
                   [embw[w * WIN:(w + 1) * WIN] for w in range(NW)], acc_d1)
            phase2(1, acc_d1)
            # zero pad rows of h1T (nodes 12500..12543) used by L2's Wr term
            nc.vector.memset(h1T[:, NB - 1, 84:128], 0.0)
            # ---- layer 2 + out ----
            phase1(g2, D2, gi2, si2, [hq_out[q][:] for q in range(4)], acc_d2)
            phase2(2, acc_d2)

    nc.compile()

    in_maps = []
    for k in range(P):
        in_maps.append({
            "embw": emb_hc, "h0o": h0_own[k],
            "gi1": gidx1[k], "si1": sidx1[k],
            "gi2": gidx2[k], "si2": sidx2[k],
            "invc": invc[k],
            "wl1": Wl1, "wr1": Wr1, "wl2": Wl2, "wr2": Wr2, "wout": Wout,
            "bl1t": bl1.reshape(HID, 1), "bl2t": bl2.reshape(HID, 1),
            "boutr": np.tile(bout.reshape(1, HID), (128, 1)),
        })
    res = run_bass_kernel_spmd(nc, in_maps, list(range(P)))
    out = np.zeros((N, HID), np.float32)
    for k in range(P):
        out[k * NPC:(k + 1) * NPC] = res.results[k]["out"][:NPC]
    kernel.last_exec_time_ns = res.exec_time_ns
    return out


# revision 22
# speedup vs baseline: 1.1552x; 1.0400x over previous
"""GNN (2x SAGEConv + linear) Bass kernel for trn2, 8 NeuronCores.

Sharding: nodes partitioned across 8 cores (12500 each, dst-range).
Each layer: per-core windowed padded-CSR gathers of h[src] (dma_gather over
4 SWDGE queues), on-chip segment reduce (DVE strided), batched unique-row
dma_scatter_add into per-window DRAM accumulators, dense combine + PE MLP.
The inter-layer AllGather of h1 slices is split into 4 quarter collectives
that pipeline with phase 2 and with layer 2's first window gathers (layer-2
gather windows are quarter-of-every-core tensors).
"""
import numpy as np

N = 100000
E = 1250000
HID = 64
P = 8
NPC = 12500          # nodes per core
RPC = 12544          # rows per core block (98 * 128), rows 12500+ are zero pads
NB = RPC // 128      # 98 blocks
WIN = 2 * RPC        # 25088 rows per L1 gather window (2 emb blocks)
NW = 4               # windows per layer
ZLOC = 12500         # local row inside an L1 window that is guaranteed zero
ACCR = RPC + 128     # accumulator rows (tail rows are scratch)
MAXPOS = 2048        # max gather positions per call
MAXT = 16            # max tiles per gather call

# layer-2 quarter windows: per-core h1 slice split at 128-node block bounds
QOFF = [0, 3072, 6400, 9472]           # node offsets within a core slice
QSZ = [3072, 3328, 3072, 3072]         # real rows per quarter
QSZP = [q + 16 for q in QSZ]           # +16 zero pad rows per core
QB0 = [0, 24, 50, 74]                  # first 128-node block of each quarter
QB1 = [24, 50, 74, 98]                 # one-past-last block


def _wrap128(vals):
    """flat int16 stream -> [128, len/16] wrapped+replicated layout."""
    n = vals.shape[0]
    w16 = np.ascontiguousarray(vals.reshape(n // 16, 16).T)
    return np.tile(w16, (8, 1))


def _build_layer_meta(w_of, loc, dst, wrows, zpad):
    """Per-layer gather/scatter metadata.

    w_of: window of each edge's source [E]
    loc:  row of the source within its window [E]
    dst:  destination node per edge [E]
    wrows: rows per window (bounds check)
    zpad: per-window row index that is guaranteed zero (pad target)
    """
    core = dst // NPC
    dstl = dst % NPC

    deg = np.zeros((P, NW, RPC), np.int32)
    np.add.at(deg, (core, w_of, dstl), 1)

    order = np.argsort(-deg, axis=2, kind="stable")  # [P, NW, RPC]
    deg_sorted = -np.sort(-deg, axis=2)
    tile_max = deg_sorted.reshape(P, NW, NB, 128).max(axis=3)
    D = tile_max.max(axis=0)                         # [NW, NB] shared

    groups = []
    for w in range(NW):
        gw = []
        cur, curpos = [], 0
        for t in range(NB):
            d = int(D[w, t])
            if d == 0:
                continue
            if cur and (curpos + d * 128 > MAXPOS or len(cur) >= MAXT):
                gw.append(cur)
                cur, curpos = [], 0
            cur.append(t)
            curpos += d * 128
        if cur:
            gw.append(cur)
        groups.append(gw)

    eorder = np.lexsort((loc, dstl, w_of, core))
    sc, sw, sd, sl = core[eorder], w_of[eorder], dstl[eorder], loc[eorder]
    key = ((sc * NW + sw) * RPC + sd).astype(np.int64)
    starts = np.searchsorted(key, np.arange(P * NW * RPC, dtype=np.int64))
    starts = np.append(starts, len(key))

    for w in range(NW):
        assert loc[w_of == w].max(initial=0) < wrows[w] <= 32767

    gidx_cores, sidx_cores = [], []
    for k in range(P):
        gparts, sparts = [], []
        for w in range(NW):
            od = order[k, w]
            for gt in groups[w]:
                for t in gt:
                    d = int(D[w, t])
                    nodes = od[t * 128:(t + 1) * 128]
                    blockg = np.full((d, 128), zpad[w], np.int32)
                    for p in range(128):
                        nloc = int(nodes[p])
                        s0 = starts[(k * NW + w) * RPC + nloc]
                        s1 = starts[(k * NW + w) * RPC + nloc + 1]
                        cnt = s1 - s0
                        if cnt:
                            blockg[:cnt, p] = sl[s0:s1]
                    gparts.append(blockg.reshape(-1))
                srows = np.concatenate(
                    [od[t * 128:(t + 1) * 128] for t in gt]).astype(np.int32)
                sparts.append(srows)
        gidx_cores.append(_wrap128(np.concatenate(gparts).astype(np.int16)))
        sidx_cores.append(_wrap128(np.concatenate(sparts).astype(np.int16)))
    return groups, D, gidx_cores, sidx_cores


def kernel(x, edge_index, edge_weight, emb, Wl1, bl1, Wr1, a1,
           Wl2, bl2, Wr2, a2, Wout, bout):
    import concourse.bacc as bacc
    import concourse.mybir as mybir
    import concourse.tile as tile
    from concourse.bass_utils import run_bass_kernel_spmd
    from concourse.masks import make_identity

    x = np.asarray(x).astype(np.int64)
    ei = np.asarray(edge_index).astype(np.int64)
    emb = np.asarray(emb, np.float32)
    Wl1 = np.asarray(Wl1, np.float32); Wr1 = np.asarray(Wr1, np.float32)
    Wl2 = np.asarray(Wl2, np.float32); Wr2 = np.asarray(Wr2, np.float32)
    Wout = np.asarray(Wout, np.float32)
    bl1 = np.asarray(bl1, np.float32); bl2 = np.asarray(bl2, np.float32)
    bout = np.asarray(bout, np.float32)
    a1f = float(np.asarray(a1)); a2f = float(np.asarray(a2))
    src, dst = ei[0], ei[1]

    # ---- host prep ------------------------------------------------------
    emb_hc = np.zeros((P * RPC, HID), np.float32)
    for r in range(P):
        emb_hc[r * RPC:r * RPC + NPC] = emb[r * NPC:(r + 1) * NPC]

    h0_own = np.zeros((P, RPC, HID), np.float32)
    for k in range(P):
        h0_own[k, :NPC] = emb[x[k * NPC:(k + 1) * NPC]]

    cnt = np.bincount(dst, minlength=N).astype(np.float32)
    invc = np.zeros((P, 128, NB), np.float32)
    for k in range(P):
        c = np.zeros(RPC, np.float32)
        c[:NPC] = 1.0 / np.maximum(cnt[k * NPC:(k + 1) * NPC], 1.0)
        invc[k] = c.reshape(NB, 128).T

    # L1: sources are emb rows in hcat layout (two 12544 blocks per window)
    sid1 = x[src]
    w1 = sid1 // (2 * NPC)
    loc1 = RPC * ((sid1 // NPC) % 2) + sid1 % NPC
    g1, D1, gidx1, sidx1 = _build_layer_meta(
        w1, loc1, dst, [WIN] * 4, [ZLOC] * 4)

    # L2: sources are h1 rows in quarter-window layout
    k2 = src // NPC
    r2 = src % NPC
    q2 = np.digitize(r2, QOFF[1:])               # quarter index 0..3
    qoff = np.array(QOFF)[q2]
    qszp = np.array(QSZP)[q2]
    loc2 = k2 * qszp + (r2 - qoff)
    g2, D2, gidx2, sidx2 = _build_layer_meta(
        q2, loc2, dst, [8 * s for s in QSZP], list(QSZ))

    # ---- device program -------------------------------------------------
    f32, i16 = mybir.dt.float32, mybir.dt.int16
    nc = bacc.Bacc(dynamic_dma_scratch_size=65536, num_swdge_queues=4)
    dp = nc.declare_dram_parameter
    embw = dp("embw", [P * RPC, HID], f32, isOutput=False)
    h0o = dp("h0o", [RPC, HID], f32, isOutput=False)
    gi1 = dp("gi1", list(gidx1[0].shape), i16, isOutput=False)
    si1 = dp("si1", list(sidx1[0].shape), i16, isOutput=False)
    gi2 = dp("gi2", list(gidx2[0].shape), i16, isOutput=False)
    si2 = dp("si2", list(sidx2[0].shape), i16, isOutput=False)
    invc_p = dp("invc", [128, NB], f32, isOutput=False)
    wl1_p = dp("wl1", [HID, HID], f32, isOutput=False)
    wr1_p = dp("wr1", [HID, HID], f32, isOutput=False)
    wl2_p = dp("wl2", [HID, HID], f32, isOutput=False)
    wr2_p = dp("wr2", [HID, HID], f32, isOutput=False)
    wout_p = dp("wout", [HID, HID], f32, isOutput=False)
    bl1_p = dp("bl1t", [HID, 1], f32, isOutput=False)
    bl2_p = dp("bl2t", [HID, 1], f32, isOutput=False)
    bout_p = dp("boutr", [128, HID], f32, isOutput=False)
    out_p = dp("out", [RPC, HID], f32, isOutput=True)

    acc_d1 = [nc.dram_tensor(f"acc{w}", [ACCR, HID], f32) for w in range(NW)]
    acc_d2 = [nc.dram_tensor(f"accB{w}", [ACCR, HID], f32) for w in range(NW)]
    hq_in = [nc.dram_tensor(f"hqi{q}", [QSZP[q], HID], f32) for q in range(4)]
    hq_out = [nc.dram_tensor(f"hqo{q}", [P * QSZP[q], HID], f32,
                             addr_space="Shared") for q in range(4)]

    AX = mybir.AxisListType.X
    ADD = mybir.AluOpType.add
    PRELU = mybir.ActivationFunctionType.Prelu

    with tile.TileContext(nc) as tc:
        with tc.tile_pool(name="const", bufs=1) as cpool, \
             tc.tile_pool(name="big", bufs=1) as bpool, \
             tc.tile_pool(name="gio", bufs=3) as gpool, \
             tc.tile_pool(name="ph2", bufs=2) as qpool, \
             tc.tile_pool(name="ps", bufs=1, space="PSUM") as ppool:

            ident = cpool.tile([128, 128], f32)
            make_identity(nc, ident[:])
            wl1_t = cpool.tile([HID, HID], f32); nc.sync.dma_start(wl1_t[:], wl1_p[:])
            wr1_t = cpool.tile([HID, HID], f32); nc.sync.dma_start(wr1_t[:], wr1_p[:])
            wl2_t = cpool.tile([HID, HID], f32); nc.sync.dma_start(wl2_t[:], wl2_p[:])
            wr2_t = cpool.tile([HID, HID], f32); nc.sync.dma_start(wr2_t[:], wr2_p[:])
            wout_t = cpool.tile([HID, HID], f32); nc.sync.dma_start(wout_t[:], wout_p[:])
            bl1_t = cpool.tile([HID, 1], f32); nc.sync.dma_start(bl1_t[:], bl1_p[:])
            bl2_t = cpool.tile([HID, 1], f32); nc.sync.dma_start(bl2_t[:], bl2_p[:])
            bout_t = cpool.tile([128, HID], f32); nc.sync.dma_start(bout_t[:], bout_p[:])
            invc_t = cpool.tile([128, NB], f32); nc.sync.dma_start(invc_t[:], invc_p[:])

            h1T = bpool.tile([HID, NB, 128], f32)      # h1 transposed, own nodes
            hc1_t = bpool.tile([128, NB, HID], f32)    # h1 node-major, own nodes
            zt = cpool.tile([128, HID], f32)
            nc.vector.memset(zt[:], 0.0)
            zbig = cpool.tile([128, 33, HID], f32)
            nc.vector.memset(zbig[:], 0.0)

            # zero pad rows of the quarter collective inputs (once)
            for q in range(4):
                nc.sync.dma_start(hq_in[q][QSZ[q]:QSZP[q]], zt[:16, :])

            def zero_accs(acc_d):
                for w in range(NW):
                    dstv = acc_d[w][:].rearrange("(r p) f -> p r f", p=128)
                    for c in range(3):
                        nc.sync.dma_start(dstv[:, c * 33:(c + 1) * 33, :], zbig[:])

            def phase1(groups, D, gi_p, si_p, win_aps, acc_d):
                gi_t = bpool.tile([128, gi_p.shape[1]], i16, tag="gi")
                si_t = bpool.tile([128, si_p.shape[1]], i16, tag="si")
                nc.sync.dma_start(gi_t[:], gi_p[:])
                nc.sync.dma_start(si_t[:], si_p[:])
                gcol = 0
                scol = 0
                qn = 0
                for w in range(NW):
                    win = win_aps[w]
                    for gt in groups[w]:
                        npos = int(sum(D[w, t] for t in gt)) * 128
                        ncols = npos // 128
                        nt = len(gt)
                        g_t = gpool.tile([128, MAXPOS // 128, HID], f32, tag="g")
                        r_t = gpool.tile([128, MAXT, HID], f32, tag="r")
                        nc.gpsimd.dma_gather(
                            g_t[:, :ncols, :], win, gi_t[:, gcol:gcol + npos // 16],
                            npos, npos, HID, single_packet=False,
                            queue_num=qn % 4)
                        off = 0
                        for i, t in enumerate(gt):
                            d = int(D[w, t])
                            view = g_t[:, off:off + d, :].rearrange("p d f -> p f d")
                            nc.vector.tensor_reduce(r_t[:, i, :], view, axis=AX, op=ADD)
                            off += d
                        nc.gpsimd.dma_scatter_add(
                            acc_d[w][:], r_t[:, :nt, :], si_t[:, scol:scol + nt * 8],
                            nt * 128, nt * 128, HID, single_packet=False,
                            queue_num=(qn + 2) % 4)
                        gcol += npos // 16
                        scol += nt * 8
                        qn += 1

            def phase2(L, acc_d):
                wl_t = wl1_t if L == 1 else wl2_t
                wr_t = wr1_t if L == 1 else wr2_t
                bl_t = bl1_t if L == 1 else bl2_t
                alpha = a1f if L == 1 else a2f
                # chunk boundaries: multiples of 4 blocks, split at quarter ends
                bounds = []
                for q in range(4):
                    b = QB0[q]
                    while b < QB1[q]:
                        nb = min(4, QB1[q] - b)
                        bounds.append((b, nb))
                        b += nb
                for (b, nb) in bounds:
                    nn = nb * 128
                    m_t = qpool.tile([128, 4, NW, HID], f32, tag="m")
                    for w in range(NW):
                        nc.sync.dma_start(
                            m_t[:, :nb, w, :],
                            acc_d[w][b * 128:b * 128 + nn].rearrange(
                                "(r p) f -> p r f", p=128))
                    mean0 = qpool.tile([128, 4, HID], f32, tag="mean0")
                    nc.vector.tensor_reduce(
                        mean0[:, :nb, :],
                        m_t[:, :nb, :, :].rearrange("p r w f -> p r f w"),
                        axis=AX, op=ADD)
                    meansc = qpool.tile([128, 4, HID], f32, tag="mean0")
                    for r in range(nb):
                        nc.vector.tensor_scalar_mul(
                            meansc[:, r, :], mean0[:, r, :],
                            invc_t[:, b + r:b + r + 1])
                    psA = ppool.tile([HID, 512], f32, tag="psA")
                    for r in range(nb):
                        nc.tensor.transpose(psA[:, r * 128:(r + 1) * 128],
                                            meansc[:, r, :], ident[:])
                    meanT = qpool.tile([HID, 512], f32, tag="meanT")
                    nc.vector.tensor_copy(meanT[:, :nn], psA[:, :nn])
                    if L == 1:
                        hob = qpool.tile([128, 4, HID], f32, tag="hob")
                        nc.sync.dma_start(
                            hob[:, :nb, :],
                            h0o[b * 128:b * 128 + nn].rearrange(
                                "(r p) f -> p r f", p=128))
                        psB = ppool.tile([HID, 512], f32, tag="psB")
                        for r in range(nb):
                            nc.tensor.transpose(psB[:, r * 128:(r + 1) * 128],
                                                hob[:, r, :], ident[:])
                        hT = qpool.tile([HID, 512], f32, tag="hT")
                        nc.vector.tensor_copy(hT[:, :nn], psB[:, :nn])
                        hT_ap = hT[:, :nn]
                    else:
                        hT_ap = h1T[:, b:b + nb, :].rearrange("f r p -> f (r p)")
                    psC = ppool.tile([HID, 512], f32, tag="psC")
                    nc.tensor.matmul(psC[:, :nn], wl_t[:], meanT[:, :nn],
                                     start=True, stop=False)
                    nc.tensor.matmul(psC[:, :nn], wr_t[:], hT_ap,
                                     start=False, stop=True)
                    if L == 1:
                        nc.scalar.activation(
                            h1T[:, b:b + nb, :].rearrange("f r p -> f (r p)"),
                            psC[:, :nn], PRELU, bias=bl_t[:], alpha=alpha)
                        psD = ppool.tile([128, 4, HID], f32, tag="psD")
                        for r in range(nb):
                            nc.tensor.transpose(psD[:, r, :], h1T[:, b + r, :],
                                                ident[:HID, :HID])
                        nc.vector.tensor_copy(hc1_t[:, b:b + nb, :],
                                              psD[:, :nb, :])
                        for q in range(4):
                            if b + nb == QB1[q]:
                                nc.sync.dma_start(
                                    hq_in[q][0:QSZ[q]].rearrange(
                                        "(r p) f -> p r f", p=128),
                                    hc1_t[:, QB0[q]:QB1[q], :])
                                if q == 3:
                                    # zero h1 pad rows (nodes 12500..12543)
                                    nc.sync.dma_start(
                                        hq_in[3][NPC - QOFF[3]:QSZ[3]],
                                        zt[:QSZ[3] - (NPC - QOFF[3]), :])
                                nc.gpsimd.collective_compute(
                                    "AllGather", mybir.AluOpType.bypass,
                                    replica_groups=[list(range(P))],
                                    ins=[hq_in[q][:]], outs=[hq_out[q][:]])
                    else:
                        h2T = qpool.tile([HID, 512], f32, tag="h2T")
                        nc.scalar.activation(h2T[:, :nn], psC[:, :nn], PRELU,
                                             bias=bl_t[:], alpha=alpha)
                        for r in range(nb):
                            psE = ppool.tile([128, HID], f32, tag="psE")
                            nc.tensor.matmul(psE[:], h2T[:, (b + r - b) * 0 + r * 128:(r + 1) * 128],
                                             wout_t[:], start=True, stop=True)
                            outb = qpool.tile([128, HID], f32, tag="outb")
                            nc.vector.tensor_tensor(outb[:], psE[:], bout_t[:], op=ADD)
                            nc.sync.dma_start(
                                out_p[(b + r) * 128:(b + r + 1) * 128], outb[:])

            # ---- layer 1 ----
            zero_accs(acc_d1)
            zero_accs(acc_d2)
            phase1(g1, D1, gi1, si1,
                   [embw[w * WIN:(w + 1) * WIN] for w in range(NW)], acc_d1)
            phase2(1, acc_d1)
            # zero pad rows of h1T (nodes 12500..12543) used by L2's Wr term
            nc.vector.memset(h1T[:, NB - 1, 84:128], 0.0)
            # ---- layer 2 + out ----
            phase1(g2, D2, gi2, si2, [hq_out[q][:] for q in range(4)], acc_d2)
            phase2(2, acc_d2)

    nc.compile()

    in_maps = []
    for k in range(P):
        in_maps.append({
            "embw": emb_hc, "h0o": h0_own[k],
            "gi1": gidx1[k], "si1": sidx1[k],
            "gi2": gidx2[k], "si2": sidx2[k],
            "invc": invc[k],
            "wl1": Wl1, "wr1": Wr1, "wl2": Wl2, "wr2": Wr2, "wout": Wout,
            "bl1t": bl1.reshape(HID, 1), "bl2t": bl2.reshape(HID, 1),
            "boutr": np.tile(bout.reshape(1, HID), (128, 1)),
        })
    res = run_bass_kernel_spmd(nc, in_maps, list(range(P)))
    out = np.zeros((N, HID), np.float32)
    for k in range(P):
        out[k * NPC:(k + 1) * NPC] = res.results[k]["out"][:NPC]
    kernel.last_exec_time_ns = res.exec_time_ns
    return out


# revision 23
# speedup vs baseline: 1.3789x; 1.1936x over previous
"""GNN (2x SAGEConv + linear) Bass kernel for trn2, 8 NeuronCores.

Sharding: nodes partitioned across 8 cores (12500 each, dst-range).
Each layer: per-core windowed padded-CSR gathers of h[src] (dma_gather over
4 SWDGE queues), on-chip segment reduce (DVE strided), batched unique-row
dma_scatter_add into per-window DRAM accumulators, dense combine + PE MLP.
The inter-layer AllGather of h1 slices is split into 4 quarter collectives
that pipeline with phase 2 and with layer 2's first window gathers (layer-2
gather windows are quarter-of-every-core tensors).
"""
import numpy as np

N = 100000
E = 1250000
HID = 64
P = 8
NPC = 12500          # nodes per core
RPC = 12544          # rows per core block (98 * 128), rows 12500+ are zero pads
NB = RPC // 128      # 98 blocks
WIN = 2 * RPC        # 25088 rows per L1 gather window (2 emb blocks)
NW = 4               # windows per layer
ZLOC = 12500         # local row inside an L1 window that is guaranteed zero
ACCR = RPC + 128     # accumulator rows (tail rows are scratch)
MAXPOS = 2048        # max gather positions per call
MAXT = 16            # max tiles per gather call

# layer-2 quarter windows: per-core h1 slice split at 128-node block bounds
QOFF = [0, 3072, 6400, 9472]           # node offsets within a core slice
QSZ = [3072, 3328, 3072, 3072]         # real rows per quarter
QSZP = [q + 16 for q in QSZ]           # +16 zero pad rows per core
QB0 = [0, 24, 50, 74]                  # first 128-node block of each quarter
QB1 = [24, 50, 74, 98]                 # one-past-last block


def _wrap128(vals):
    """flat int16 stream -> [128, len/16] wrapped+replicated layout."""
    n = vals.shape[0]
    w16 = np.ascontiguousarray(vals.reshape(n // 16, 16).T)
    return np.tile(w16, (8, 1))


def _build_layer_meta(w_of, loc, dst, wrows, zpad):
    """Per-layer gather/scatter metadata.

    w_of: window of each edge's source [E]
    loc:  row of the source within its window [E]
    dst:  destination node per edge [E]
    wrows: rows per window (bounds check)
    zpad: per-window row index that is guaranteed zero (pad target)
    """
    core = dst // NPC
    dstl = dst % NPC

    deg = np.zeros((P, NW, RPC), np.int32)
    np.add.at(deg, (core, w_of, dstl), 1)

    order = np.argsort(-deg, axis=2, kind="stable")  # [P, NW, RPC]
    deg_sorted = -np.sort(-deg, axis=2)
    tile_max = deg_sorted.reshape(P, NW, NB, 128).max(axis=3)
    D = tile_max.max(axis=0)                         # [NW, NB] shared

    groups = []
    for w in range(NW):
        gw = []
        cur, curpos = [], 0
        for t in range(NB):
            d = int(D[w, t])
            if d == 0:
                continue
            if cur and (curpos + d * 128 > MAXPOS or len(cur) >= MAXT):
                gw.append(cur)
                cur, curpos = [], 0
            cur.append(t)
            curpos += d * 128
        if cur:
            gw.append(cur)
        groups.append(gw)

    eorder = np.lexsort((loc, dstl, w_of, core))
    sc, sw, sd, sl = core[eorder], w_of[eorder], dstl[eorder], loc[eorder]
    key = ((sc * NW + sw) * RPC + sd).astype(np.int64)
    starts = np.searchsorted(key, np.arange(P * NW * RPC, dtype=np.int64))
    starts = np.append(starts, len(key))

    for w in range(NW):
        assert loc[w_of == w].max(initial=0) < wrows[w] <= 32767

    gidx_cores, sidx_cores = [], []
    for k in range(P):
        gparts, sparts = [], []
        for w in range(NW):
            od = order[k, w]
            for gt in groups[w]:
                for t in gt:
                    d = int(D[w, t])
                    nodes = od[t * 128:(t + 1) * 128]
                    blockg = np.full((d, 128), zpad[w], np.int32)
                    for p in range(128):
                        nloc = int(nodes[p])
                        s0 = starts[(k * NW + w) * RPC + nloc]
                        s1 = starts[(k * NW + w) * RPC + nloc + 1]
                        cnt = s1 - s0
                        if cnt:
                            blockg[:cnt, p] = sl[s0:s1]
                    gparts.append(blockg.reshape(-1))
                srows = np.concatenate(
                    [od[t * 128:(t + 1) * 128] for t in gt]).astype(np.int32)
                sparts.append(srows)
        gidx_cores.append(_wrap128(np.concatenate(gparts).astype(np.int16)))
        sidx_cores.append(_wrap128(np.concatenate(sparts).astype(np.int16)))
    return groups, D, gidx_cores, sidx_cores


def kernel(x, edge_index, edge_weight, emb, Wl1, bl1, Wr1, a1,
           Wl2, bl2, Wr2, a2, Wout, bout):
    import concourse.bacc as bacc
    import concourse.mybir as mybir
    import concourse.tile as tile
    from concourse.bass_utils import run_bass_kernel_spmd
    from concourse.masks import make_identity

    x = np.asarray(x).astype(np.int64)
    ei = np.asarray(edge_index).astype(np.int64)
    emb = np.asarray(emb, np.float32)
    Wl1 = np.asarray(Wl1, np.float32); Wr1 = np.asarray(Wr1, np.float32)
    Wl2 = np.asarray(Wl2, np.float32); Wr2 = np.asarray(Wr2, np.float32)
    Wout = np.asarray(Wout, np.float32)
    bl1 = np.asarray(bl1, np.float32); bl2 = np.asarray(bl2, np.float32)
    bout = np.asarray(bout, np.float32)
    a1f = float(np.asarray(a1)); a2f = float(np.asarray(a2))
    src, dst = ei[0], ei[1]

    # ---- host prep ------------------------------------------------------
    emb_hc = np.zeros((P * RPC, HID), np.float32)
    for r in range(P):
        emb_hc[r * RPC:r * RPC + NPC] = emb[r * NPC:(r + 1) * NPC]

    h0_own = np.zeros((P, RPC, HID), np.float32)
    for k in range(P):
        h0_own[k, :NPC] = emb[x[k * NPC:(k + 1) * NPC]]

    cnt = np.bincount(dst, minlength=N).astype(np.float32)
    invc = np.zeros((P, 128, NB), np.float32)
    for k in range(P):
        c = np.zeros(RPC, np.float32)
        c[:NPC] = 1.0 / np.maximum(cnt[k * NPC:(k + 1) * NPC], 1.0)
        invc[k] = c.reshape(NB, 128).T

    # L1: sources are emb rows in hcat layout (two 12544 blocks per window)
    sid1 = x[src]
    w1 = sid1 // (2 * NPC)
    loc1 = RPC * ((sid1 // NPC) % 2) + sid1 % NPC
    g1, D1, gidx1, sidx1 = _build_layer_meta(
        w1, loc1, dst, [WIN] * 4, [ZLOC] * 4)

    # L2: sources are h1 rows in quarter-window layout
    k2 = src // NPC
    r2 = src % NPC
    q2 = np.digitize(r2, QOFF[1:])               # quarter index 0..3
    qoff = np.array(QOFF)[q2]
    qszp = np.array(QSZP)[q2]
    loc2 = k2 * qszp + (r2 - qoff)
    g2, D2, gidx2, sidx2 = _build_layer_meta(
        q2, loc2, dst, [8 * s for s in QSZP], list(QSZ))

    # ---- device program -------------------------------------------------
    f32, i16 = mybir.dt.float32, mybir.dt.int16
    nc = bacc.Bacc(dynamic_dma_scratch_size=65536, num_swdge_queues=4)
    dp = nc.declare_dram_parameter
    embw = dp("embw", [P * RPC, HID], f32, isOutput=False)
    h0o = dp("h0o", [RPC, HID], f32, isOutput=False)
    gi1 = dp("gi1", list(gidx1[0].shape), i16, isOutput=False)
    si1 = dp("si1", list(sidx1[0].shape), i16, isOutput=False)
    gi2 = dp("gi2", list(gidx2[0].shape), i16, isOutput=False)
    si2 = dp("si2", list(sidx2[0].shape), i16, isOutput=False)
    invc_p = dp("invc", [128, NB], f32, isOutput=False)
    wl1_p = dp("wl1", [HID, HID], f32, isOutput=False)
    wr1_p = dp("wr1", [HID, HID], f32, isOutput=False)
    wl2_p = dp("wl2", [HID, HID], f32, isOutput=False)
    wr2_p = dp("wr2", [HID, HID], f32, isOutput=False)
    wout_p = dp("wout", [HID, HID], f32, isOutput=False)
    bl1_p = dp("bl1t", [HID, 1], f32, isOutput=False)
    bl2_p = dp("bl2t", [HID, 1], f32, isOutput=False)
    bout_p = dp("boutr", [128, HID], f32, isOutput=False)
    out_p = dp("out", [RPC, HID], f32, isOutput=True)

    acc_d1 = [nc.dram_tensor(f"acc{w}", [ACCR, HID], f32) for w in range(NW)]
    acc_d2 = [nc.dram_tensor(f"accB{w}", [ACCR, HID], f32) for w in range(NW)]
    hq_in = [nc.dram_tensor(f"hqi{q}", [QSZP[q], HID], f32) for q in range(4)]
    hq_out = [nc.dram_tensor(f"hqo{q}", [P * QSZP[q], HID], f32,
                             addr_space="Shared") for q in range(4)]

    AX = mybir.AxisListType.X
    ADD = mybir.AluOpType.add
    PRELU = mybir.ActivationFunctionType.Prelu

    with tile.TileContext(nc) as tc:
        with tc.tile_pool(name="const", bufs=1) as cpool, \
             tc.tile_pool(name="big", bufs=1) as bpool, \
             tc.tile_pool(name="gio", bufs=3) as gpool, \
             tc.tile_pool(name="ph2", bufs=2) as qpool, \
             tc.tile_pool(name="ps", bufs=1, space="PSUM") as ppool:

            ident = cpool.tile([128, 128], f32)
            make_identity(nc, ident[:])
            wl1_t = cpool.tile([HID, HID], f32); nc.sync.dma_start(wl1_t[:], wl1_p[:])
            wr1_t = cpool.tile([HID, HID], f32); nc.sync.dma_start(wr1_t[:], wr1_p[:])
            wl2_t = cpool.tile([HID, HID], f32); nc.sync.dma_start(wl2_t[:], wl2_p[:])
            wr2_t = cpool.tile([HID, HID], f32); nc.sync.dma_start(wr2_t[:], wr2_p[:])
            wout_t = cpool.tile([HID, HID], f32); nc.sync.dma_start(wout_t[:], wout_p[:])
            bl1_t = cpool.tile([HID, 1], f32); nc.sync.dma_start(bl1_t[:], bl1_p[:])
            bl2_t = cpool.tile([HID, 1], f32); nc.sync.dma_start(bl2_t[:], bl2_p[:])
            bout_t = cpool.tile([128, HID], f32); nc.sync.dma_start(bout_t[:], bout_p[:])
            invc_t = cpool.tile([128, NB], f32); nc.sync.dma_start(invc_t[:], invc_p[:])

            h1T = bpool.tile([HID, NB, 128], f32)      # h1 transposed, own nodes
            hc1_t = bpool.tile([128, NB, HID], f32)    # h1 node-major, own nodes
            zt = cpool.tile([128, HID], f32)
            nc.vector.memset(zt[:], 0.0)
            zbig = cpool.tile([128, 33, HID], f32)
            nc.vector.memset(zbig[:], 0.0)

            # zero pad rows of the quarter collective inputs (once)
            for q in range(4):
                nc.sync.dma_start(hq_in[q][QSZ[q]:QSZP[q]], zt[:16, :])

            def zero_accs(acc_d):
                for w in range(NW):
                    dstv = acc_d[w][:].rearrange("(r p) f -> p r f", p=128)
                    for c in range(3):
                        nc.sync.dma_start(dstv[:, c * 33:(c + 1) * 33, :], zbig[:])

            def phase1(groups, D, gi_p, si_p, win_aps, acc_d):
                gi_t = bpool.tile([128, gi_p.shape[1]], i16, tag="gi")
                si_t = bpool.tile([128, si_p.shape[1]], i16, tag="si")
                nc.sync.dma_start(gi_t[:], gi_p[:])
                nc.sync.dma_start(si_t[:], si_p[:])
                gcol = 0
                scol = 0
                qn = 0
                for w in range(NW):
                    win = win_aps[w]
                    for gt in groups[w]:
                        npos = int(sum(D[w, t] for t in gt)) * 128
                        ncols = npos // 128
                        nt = len(gt)
                        g_t = gpool.tile([128, MAXPOS // 128, HID], f32, tag="g")
                        r_t = gpool.tile([128, MAXT, HID], f32, tag="r")
                        nc.gpsimd.dma_gather(
                            g_t[:, :ncols, :], win, gi_t[:, gcol:gcol + npos // 16],
                            npos, npos, HID, single_packet=False,
                            queue_num=qn % 3)
                        off = 0
                        for i, t in enumerate(gt):
                            d = int(D[w, t])
                            view = g_t[:, off:off + d, :].rearrange("p d f -> p f d")
                            nc.vector.tensor_reduce(r_t[:, i, :], view, axis=AX, op=ADD)
                            off += d
                        nc.gpsimd.dma_scatter_add(
                            acc_d[w][:], r_t[:, :nt, :], si_t[:, scol:scol + nt * 8],
                            nt * 128, nt * 128, HID, single_packet=False,
                            queue_num=3)
                        gcol += npos // 16
                        scol += nt * 8
                        qn += 1

            def phase2(L, acc_d):
                wl_t = wl1_t if L == 1 else wl2_t
                wr_t = wr1_t if L == 1 else wr2_t
                bl_t = bl1_t if L == 1 else bl2_t
                alpha = a1f if L == 1 else a2f
                # chunk boundaries: multiples of 4 blocks, split at quarter ends
                bounds = []
                for q in range(4):
                    b = QB0[q]
                    while b < QB1[q]:
                        nb = min(4, QB1[q] - b)
                        bounds.append((b, nb))
                        b += nb
                for (b, nb) in bounds:
                    nn = nb * 128
                    m_t = qpool.tile([128, 4, NW, HID], f32, tag="m")
                    for w in range(NW):
                        nc.sync.dma_start(
                            m_t[:, :nb, w, :],
                            acc_d[w][b * 128:b * 128 + nn].rearrange(
                                "(r p) f -> p r f", p=128))
                    mean0 = qpool.tile([128, 4, HID], f32, tag="mean0")
                    nc.vector.tensor_reduce(
                        mean0[:, :nb, :],
                        m_t[:, :nb, :, :].rearrange("p r w f -> p r f w"),
                        axis=AX, op=ADD)
                    meansc = qpool.tile([128, 4, HID], f32, tag="mean0")
                    for r in range(nb):
                        nc.vector.tensor_scalar_mul(
                            meansc[:, r, :], mean0[:, r, :],
                            invc_t[:, b + r:b + r + 1])
                    psA = ppool.tile([HID, 512], f32, tag="psA")
                    for r in range(nb):
                        nc.tensor.transpose(psA[:, r * 128:(r + 1) * 128],
                                            meansc[:, r, :], ident[:])
                    meanT = qpool.tile([HID, 512], f32, tag="meanT")
                    nc.vector.tensor_copy(meanT[:, :nn], psA[:, :nn])
                    if L == 1:
                        hob = qpool.tile([128, 4, HID], f32, tag="hob")
                        nc.sync.dma_start(
                            hob[:, :nb, :],
                            h0o[b * 128:b * 128 + nn].rearrange(
                                "(r p) f -> p r f", p=128))
                        psB = ppool.tile([HID, 512], f32, tag="psB")
                        for r in range(nb):
                            nc.tensor.transpose(psB[:, r * 128:(r + 1) * 128],
                                                hob[:, r, :], ident[:])
                        hT = qpool.tile([HID, 512], f32, tag="hT")
                        nc.vector.tensor_copy(hT[:, :nn], psB[:, :nn])
                        hT_ap = hT[:, :nn]
                    else:
                        hT_ap = h1T[:, b:b + nb, :].rearrange("f r p -> f (r p)")
                    psC = ppool.tile([HID, 512], f32, tag="psC")
                    nc.tensor.matmul(psC[:, :nn], wl_t[:], meanT[:, :nn],
                                     start=True, stop=False)
                    nc.tensor.matmul(psC[:, :nn], wr_t[:], hT_ap,
                                     start=False, stop=True)
                    if L == 1:
                        nc.scalar.activation(
                            h1T[:, b:b + nb, :].rearrange("f r p -> f (r p)"),
                            psC[:, :nn], PRELU, bias=bl_t[:], alpha=alpha)
                        psD = ppool.tile([128, 4, HID], f32, tag="psD")
                        for r in range(nb):
                            nc.tensor.transpose(psD[:, r, :], h1T[:, b + r, :],
                                                ident[:HID, :HID])
                        nc.vector.tensor_copy(hc1_t[:, b:b + nb, :],
                                              psD[:, :nb, :])
                        for q in range(4):
                            if b + nb == QB1[q]:
                                nc.sync.dma_start(
                                    hq_in[q][0:QSZ[q]].rearrange(
                                        "(r p) f -> p r f", p=128),
                                    hc1_t[:, QB0[q]:QB1[q], :])
                                if q == 3:
                                    # zero h1 pad rows (nodes 12500..12543)
                                    nc.sync.dma_start(
                                        hq_in[3][NPC - QOFF[3]:QSZ[3]],
                                        zt[:QSZ[3] - (NPC - QOFF[3]), :])
                                nc.gpsimd.collective_compute(
                                    "AllGather", mybir.AluOpType.bypass,
                                    replica_groups=[list(range(P))],
                                    ins=[hq_in[q][:]], outs=[hq_out[q][:]])
                    else:
                        h2T = qpool.tile([HID, 512], f32, tag="h2T")
                        nc.scalar.activation(h2T[:, :nn], psC[:, :nn], PRELU,
                                             bias=bl_t[:], alpha=alpha)
                        for r in range(nb):
                            psE = ppool.tile([128, HID], f32, tag="psE")
                            nc.tensor.matmul(psE[:], h2T[:, (b + r - b) * 0 + r * 128:(r + 1) * 128],
                                             wout_t[:], start=True, stop=True)
                            outb = qpool.tile([128, HID], f32, tag="outb")
                            nc.vector.tensor_tensor(outb[:], psE[:], bout_t[:], op=ADD)
                            nc.sync.dma_start(
                                out_p[(b + r) * 128:(b + r + 1) * 128], outb[:])

            # ---- layer 1 ----
            zero_accs(acc_d1)
            zero_accs(acc_d2)
            phase1(g1, D1, gi1, si1,
                   [embw[w * WIN:(w + 1) * WIN] for w in range(NW)], acc_d1)
            phase2(1, acc_d1)
            # zero pad rows of h1T (nodes 12500..12543) used by L2's Wr term
            nc.vector.memset(h1T[:, NB - 1, 84:128], 0.0)
            # ---- layer 2 + out ----
            phase1(g2, D2, gi2, si2, [hq_out[q][:] for q in range(4)], acc_d2)
            phase2(2, acc_d2)

    nc.compile()

    in_maps = []
    for k in range(P):
        in_maps.append({
            "embw": emb_hc, "h0o": h0_own[k],
            "gi1": gidx1[k], "si1": sidx1[k],
            "gi2": gidx2[k], "si2": sidx2[k],
            "invc": invc[k],
            "wl1": Wl1, "wr1": Wr1, "wl2": Wl2, "wr2": Wr2, "wout": Wout,
            "bl1t": bl1.reshape(HID, 1), "bl2t": bl2.reshape(HID, 1),
            "boutr": np.tile(bout.reshape(1, HID), (128, 1)),
        })
    res = run_bass_kernel_spmd(nc, in_maps, list(range(P)))
    out = np.zeros((N, HID), np.float32)
    for k in range(P):
        out[k * NPC:(k + 1) * NPC] = res.results[k]["out"][:NPC]
    kernel.last_exec_time_ns = res.exec_time_ns
    return out
